# revision 14
# baseline (speedup 1.0000x reference)
"""Radix-2 Trainium2 kernel for CoherentDONN (v4).

A = (F2^H (x) I_256) . blockdiag(B2_0, B2_1) . (F2 (x) I_256)  (exact:
the circulant A commutes with shift-by-256).  All DFT2 combines have
REAL +/-1 coefficients - no re/im mixing anywhere in the glue.

Per layer: pre1 (4 paired wide adds) -> S1 (64 fp16 matmuls/img, N=256)
-> pre2 (8 psum adds/img) -> S2 -> post-n (8 psum adds/img) -> post-m
(4 paired adds) -> mask (6 paired muls w/ broadcast masks).
Glue ops process image PAIRS in one instruction where possible.

v4: the dispatch tunnel re-uploads every input buffer per call, so the
input footprint dominates measured time.  x, phases and fc_w ship as
int8 (dequantized on device: Act casts, sin/cos of the phase masks via
the Sin activation, per-class fc scales folded in at the end); m2imn is
negated on device.  FC runs as one 2048-matmul chain at the end instead
of 8192 interleaved tiny matmuls.
"""

import os
import numpy as np

import concourse.bass as bass
import concourse.mybir as mybir
import concourse.tile as tile
from concourse import bacc

N_CORES = int(os.environ.get("DONN_CORES", "8"))
PER_CORE = int(os.environ.get("DONN_IMG", str(128 // max(N_CORES, 1))))
RES = 512
NB = 128
NL = 3
NCLS = 10
LAMBDA = 5.32e-07
Z = 0.035
DX = 1e-06

f32 = mybir.dt.float32
f16 = mybir.dt.float16
i8 = mybir.dt.int8
MULT = mybir.AluOpType.mult
ADD = mybir.AluOpType.add
SUB = mybir.AluOpType.subtract
SQUARE = mybir.ActivationFunctionType.Square
COPY = mybir.ActivationFunctionType.Copy
SIN = mybir.ActivationFunctionType.Sin


def _make_A():
    fx = np.fft.fftfreq(RES, DX)
    h = np.exp(-1j * np.pi * LAMBDA * Z * fx**2)
    F = np.fft.fft(np.eye(RES))
    return F.conj().T @ np.diag(h) @ F / RES


def _host_constants():
    A = _make_A()
    F2 = np.array([[1, 1], [1, -1]], dtype=complex)
    T2 = np.kron(F2, np.eye(256))
    B2 = T2 @ A @ T2.conj().T / 4.0
    Bj = [B2[:256, :256], B2[256:, 256:]]
    # moving planes m2??[p, j, s, b] = Bj[j].T[s*128+p, b]
    m2re = np.zeros((NB, 2, 2, 256), np.float32)
    m2im = np.zeros((NB, 2, 2, 256), np.float32)
    for j in range(2):
        BT = Bj[j].T
        for s in range(2):
            m2re[:, j, s, :] = BT.real[s*NB:(s+1)*NB, :]
            m2im[:, j, s, :] = BT.imag[s*NB:(s+1)*NB, :]
    c16 = lambda m: np.ascontiguousarray(m, np.float16)
    return c16(m2re), c16(m2im)


def _build(nc_cache={}):
    if "nc" in nc_cache:
        return nc_cache["nc"], None

    nc = bacc.Bacc("TRN2", target_bir_lowering=False, debug=False,
                   num_devices=N_CORES)

    x_d = nc.dram_tensor("x", [PER_CORE, NB, 4, RES], i8, kind="ExternalInput").ap()
    scl_d = nc.dram_tensor("scl", [NB, 2], f32, kind="ExternalInput").ap()
    m2re_d = nc.dram_tensor("m2re", [NB, 2, 2, 256], f16, kind="ExternalInput").ap()
    m2im_d = nc.dram_tensor("m2im", [NB, 2, 2, 256], f16, kind="ExternalInput").ap()
    ph_d = nc.dram_tensor("ph8", [NL, NB, 4, RES], i8, kind="ExternalInput").ap()
    w8_d = nc.dram_tensor("w8", [NB, NCLS, 4 * RES], i8, kind="ExternalInput").ap()
    scq_d = nc.dram_tensor("scq", [PER_CORE, NCLS], f32, kind="ExternalInput").ap()
    fcb_d = nc.dram_tensor("fcb", [PER_CORE, NCLS], f32, kind="ExternalInput").ap()
    out_d = nc.dram_tensor("out", [PER_CORE, NCLS], f32, kind="ExternalOutput").ap()

    with tile.TileContext(nc) as tc:
        with tc.tile_pool(name="consts", bufs=1) as constp, \
             tc.tile_pool(name="dram", bufs=1, space="DRAM") as dramp:
            scl_t = constp.tile([NB, 2], f32, tag="scl")
            nc.sync.dma_start(scl_t[:], scl_d[:])
            pcos, psin = [], []
            for l in range(NL):
                ct = constp.tile([NB, 4, RES], f16, tag=f"pc{l}")
                st = constp.tile([NB, 4, RES], f16, tag=f"ps{l}")
                pcos.append(ct)
                psin.append(st)
            # one shared staging tile for the int8 phases (WAR deps
            # serialize the three mask builds; they are tiny)
            ph8 = [constp.tile([NB, 4, RES], i8, tag="ph8", bufs=1,
                               name=f"ph8_{l}")
                   for l in range(NL)]
            sph = scl_t[:, 1:2]
            halfpi = constp.tile([NB, 1], f32, tag="halfpi")
            nc.gpsimd.memset(halfpi[:], float(np.pi / 2))

            def make_masks(l):
                nc.sync.dma_start(ph8[l][:], ph_d[l])
                nc.scalar.activation(pcos[l][:], ph8[l][:], SIN,
                                     bias=halfpi[:], scale=sph)
                nc.scalar.activation(psin[l][:], ph8[l][:], SIN,
                                     bias=0.0, scale=sph)
            # layer-0 masks first so entry can start immediately
            make_masks(0)
            m2re = constp.tile([NB, 2, 2, 256], f16, tag="m2re")
            m2im = constp.tile([NB, 2, 2, 256], f16, tag="m2im")
            m2imn = constp.tile([NB, 2, 2, 256], f16, tag="m2imn")
            fcb_t = constp.tile([PER_CORE, NCLS], f32, tag="fcb")
            scq_t = constp.tile([PER_CORE, NCLS], f32, tag="scq")

            def late_const_dmas():
                for t, d in ((m2re, m2re_d), (m2im, m2im_d)):
                    nc.sync.dma_start(t[:], d[:])
                nc.vector.tensor_scalar_mul(m2imn[:], m2im[:], -1.0)
                for l in range(1, NL):
                    make_masks(l)
                nc.sync.dma_start(fcb_t[:], fcb_d[:])
                nc.sync.dma_start(scq_t[:], scq_d[:])
            featd = dramp.tile([NB, PER_CORE, 4 * RES], f16)

            def bc(ap):
                """broadcast a [128, 4, 512] const AP across the img axis."""
                return ap.unsqueeze(1).broadcast_to([NB, 2, 4, RES])

            with tc.tile_pool(name="xp", bufs=1) as xpool, \
                 tc.tile_pool(name="vp", bufs=2) as vpool, \
                 tc.tile_pool(name="pp", bufs=1) as ppool, \
                 tc.tile_pool(name="qp", bufs=2) as qpool, \
                 tc.tile_pool(name="up", bufs=1) as upool, \
                 tc.tile_pool(name="uu", bufs=1) as uupool, \
                 tc.tile_pool(name="sc", bufs=1) as scp, \
                 tc.tile_pool(name="fcw", bufs=2) as fcwp, \
                 tc.tile_pool(name="fcr", bufs=2) as fcrp, \
                 tc.tile_pool(name="fco", bufs=1) as fcop, \
                 tc.tile_pool(name="ps", bufs=8, space="PSUM") as psum:

                def pre1(vre, vim):
                    """paired DFT2 over c: P_j = V_lo +/- V_hi.
                    P tiles [128, 2img, 2j, 2sub, 512]."""
                    Pre = ppool.tile([NB, 2, 2, 2, RES], f16, tag="p1r")
                    Pim = ppool.tile([NB, 2, 2, 2, RES], f16, tag="p1i")
                    nc.vector.tensor_tensor(Pre[:, :, 0, :, :], vre[:, :, 0:2, :], vre[:, :, 2:4, :], ADD)
                    nc.vector.tensor_tensor(Pim[:, :, 0, :, :], vim[:, :, 0:2, :], vim[:, :, 2:4, :], ADD)
                    nc.vector.tensor_tensor(Pre[:, :, 1, :, :], vre[:, :, 0:2, :], vre[:, :, 2:4, :], SUB)
                    nc.vector.tensor_tensor(Pim[:, :, 1, :, :], vim[:, :, 0:2, :], vim[:, :, 2:4, :], SUB)
                    return Pre, Pim

                def mm_stage(slicer):
                    """One radix-2 stage; slicer(j, s, a) -> (re, im) lhsT
                    slices. Bank order 0,2,1,3 x (re,im)."""
                    banks = {}
                    for a in (0, 2, 1, 3):
                        pr = psum.tile([NB, RES], f32, tag="bk")
                        pi = psum.tile([NB, RES], f32, tag="bk")
                        for j in range(2):
                            dst = pr[:, bass.ts(j, 256)]
                            dsti = pi[:, bass.ts(j, 256)]
                            for s in range(2):
                                ls, li = slicer(j, s, a)
                                nc.tensor.matmul(dst, ls, m2re[:, j, s, :],
                                                 start=(s == 0), stop=False)
                                nc.tensor.matmul(dsti, ls, m2im[:, j, s, :],
                                                 start=(s == 0), stop=False)
                            for s in range(2):
                                ls, li = slicer(j, s, a)
                                nc.tensor.matmul(dst, li, m2imn[:, j, s, :],
                                                 start=False, stop=(s == 1))
                                nc.tensor.matmul(dsti, li, m2re[:, j, s, :],
                                                 start=False, stop=(s == 1))
                        banks[a] = (pr, pi)
                    return banks

                def drain(banks, dre, dim):
                    """Act: psum banks -> per-image fp16 tiles [128,4a,512]."""
                    for a in (0, 2, 1, 3):
                        nc.scalar.activation(dre[:, a, :], banks[a][0][:], COPY)
                        nc.scalar.activation(dim[:, a, :], banks[a][1][:], COPY)

                def img_dft2(dre, dim, dst_re, dst_im):
                    """per-image +/- combines of drained bank pairs (0,2),(1,3):
                    dst[j2][s] slices [128, 512]."""
                    for s, (lo, hi) in enumerate(((0, 2), (1, 3))):
                        nc.gpsimd.tensor_tensor(dst_re[0][s], dre[:, lo, :], dre[:, hi, :], ADD)
                        nc.vector.tensor_tensor(dst_im[0][s], dim[:, lo, :], dim[:, hi, :], ADD)
                        nc.gpsimd.tensor_tensor(dst_re[1][s], dre[:, lo, :], dre[:, hi, :], SUB)
                        nc.vector.tensor_tensor(dst_im[1][s], dim[:, lo, :], dim[:, hi, :], SUB)

                def pre2_img(P2re, P2im, dre, dim):
                    # P2 per-image [128, 2j2, 2s, 512]
                    img_dft2(dre, dim,
                             [[P2re[:, j2, s, :] for s in range(2)] for j2 in range(2)],
                             [[P2im[:, j2, s, :] for s in range(2)] for j2 in range(2)])

                def postn_img(Unre, Unim, dre, dim, im):
                    # Un paired [128, 2img, 4nc, 512], written per image
                    img_dft2(dre, dim,
                             [[Unre[:, im, 0, :], Unre[:, im, 1, :]],
                              [Unre[:, im, 2, :], Unre[:, im, 3, :]]],
                             [[Unim[:, im, 0, :], Unim[:, im, 1, :]],
                              [Unim[:, im, 2, :], Unim[:, im, 3, :]]])

                def postm(Unre, Unim):
                    Ure = uupool.tile([NB, 2, 4, RES], f16, tag="ur")
                    Uim = uupool.tile([NB, 2, 4, RES], f16, tag="ui")
                    lo = slice(0, 256)
                    hi = slice(256, 512)
                    nc.vector.tensor_tensor(Ure[:, :, :, lo], Unre[:, :, :, lo], Unre[:, :, :, hi], ADD)
                    nc.vector.tensor_tensor(Uim[:, :, :, lo], Unim[:, :, :, lo], Unim[:, :, :, hi], ADD)
                    nc.vector.tensor_tensor(Ure[:, :, :, hi], Unre[:, :, :, lo], Unre[:, :, :, hi], SUB)
                    nc.vector.tensor_tensor(Uim[:, :, :, hi], Unim[:, :, :, lo], Unim[:, :, :, hi], SUB)
                    return Ure, Uim

                def mask_mults(Ure, Uim, l):
                    t1 = scp.tile([NB, 2, 4, RES], f16, tag="t1")
                    t2 = scp.tile([NB, 2, 4, RES], f16, tag="t2")
                    c = bc(pcos[l][:])
                    s = bc(psin[l][:])
                    nc.vector.tensor_tensor(t1[:], Ure[:], c, MULT)
                    nc.gpsimd.tensor_tensor(t2[:], Uim[:], s, MULT)
                    t3 = scp.tile([NB, 2, 4, RES], f16, tag="t1")
                    t4 = scp.tile([NB, 2, 4, RES], f16, tag="t2")
                    nc.vector.tensor_tensor(t3[:], Ure[:], s, MULT)
                    nc.vector.tensor_tensor(t4[:], Uim[:], c, MULT)
                    return t1, t2, t3, t4

                def mask_combine(t1, t2, t3, t4):
                    vre = vpool.tile([NB, 2, 4, RES], f16, tag="vr")
                    vim = vpool.tile([NB, 2, 4, RES], f16, tag="vi")
                    nc.vector.tensor_tensor(vre[:], t1[:], t2[:], SUB)
                    nc.vector.tensor_tensor(vim[:], t3[:], t4[:], ADD)
                    return vre, vim

                def entry(pr_i):
                    xt = xpool.tile([NB, 2, 4, RES], f16, tag="x")
                    for im in range(2):
                        xt8 = xpool.tile([NB, 4, RES], i8, tag="x8", bufs=2,
                                         name=f"xt8_{im}")
                        nc.sync.dma_start(xt8[:], x_d[2 * pr_i + im])
                        nc.scalar.activation(xt[:, im, :, :], xt8[:], COPY,
                                             scale=scl_t[:, 0:1])
                    vre = vpool.tile([NB, 2, 4, RES], f16, tag="vr")
                    vim = vpool.tile([NB, 2, 4, RES], f16, tag="vi")
                    nc.vector.tensor_tensor(vre[:], xt[:], bc(pcos[0][:]), MULT)
                    nc.vector.tensor_tensor(vim[:], xt[:], bc(psin[0][:]), MULT)
                    return vre, vim

                def exit_intensity(Ure, Uim, pr_i):
                    s1 = scp.tile([NB, 2, 4, RES], f16, tag="t1")
                    s2 = scp.tile([NB, 2, 4, RES], f16, tag="t2")
                    nc.scalar.activation(s1[:], Ure[:], SQUARE)
                    nc.scalar.activation(s2[:], Uim[:], SQUARE)
                    ft = upool.tile([NB, 2, 2, 2, RES], f16, tag="ft")
                    nc.vector.tensor_tensor(ft[:], s1[:], s2[:], ADD)
                    nc.sync.dma_start(
                        featd[:, 2 * pr_i:2 * pr_i + 2, :].rearrange(
                            "p i f -> p (i f)"),
                        ft[:].rearrange("p i j s m -> p (i j s m)"))

                def fc_pass():
                    """One accumulation chain over all 2048 feature columns
                    for all PER_CORE images at once."""
                    NF = 4 * RES
                    CH = 64
                    ps_fc = psum.tile([NB, RES], f32, tag="bk")
                    for c in range(NF // CH):
                        wt8 = fcwp.tile([NB, NCLS, CH], i8, tag="w8")
                        nc.sync.dma_start(wt8[:], w8_d[:, :, bass.ts(c, CH)])
                        wch = fcwp.tile([NB, NCLS, CH], f16, tag="wch")
                        nc.scalar.activation(wch[:], wt8[:], COPY)
                        fch = fcrp.tile([NB, PER_CORE, CH], f16, tag="fch")
                        nc.sync.dma_start(fch[:], featd[:, :, bass.ts(c, CH)])
                        for k in range(CH):
                            f = c * CH + k
                            nc.tensor.matmul(ps_fc[0:PER_CORE, 0:NCLS],
                                             fch[:, :, k], wch[:, :, k],
                                             start=(f == 0), stop=(f == NF - 1))
                    tmp = fcop.tile([PER_CORE, NCLS], f32, tag="obt")
                    nc.vector.tensor_tensor(tmp[:], ps_fc[0:PER_CORE, 0:NCLS],
                                            scq_t[:], MULT)
                    ob = fcop.tile([PER_CORE, NCLS], f32, tag="ob")
                    nc.vector.tensor_tensor(ob[:], tmp[:], fcb_t[:], ADD)
                    nc.sync.dma_start(out_d[:], ob[:])

                def chunk_H1(st):
                    Pre, Pim = pre1(st["vre"], st["vim"])
                    P2 = []
                    for im in range(2):
                        P2re = qpool.tile([NB, 2, 2, RES], f16, tag=f"p2r{im}")
                        P2im = qpool.tile([NB, 2, 2, RES], f16, tag=f"p2i{im}")
                        dre = scp.tile([NB, 4, RES], f16, tag=f"d_r{im}")
                        dim = scp.tile([NB, 4, RES], f16, tag=f"d_i{im}")
                        bk = mm_stage(lambda j, s, a, im=im:
                                      (Pre[:, im, j, s, bass.ts(a, NB)],
                                       Pim[:, im, j, s, bass.ts(a, NB)]))
                        drain(bk, dre, dim)
                        pre2_img(P2re, P2im, dre, dim)
                        P2.append((P2re, P2im))
                    st["P2"] = P2

                def chunk_H2mm(st, l):
                    P2 = st["P2"]
                    Unre = upool.tile([NB, 2, 4, RES], f16, tag="unr")
                    Unim = upool.tile([NB, 2, 4, RES], f16, tag="uni")
                    for im in range(2):
                        P2re, P2im = P2[im]
                        dre = scp.tile([NB, 4, RES], f16, tag=f"d_r{im}")
                        dim = scp.tile([NB, 4, RES], f16, tag=f"d_i{im}")
                        bk = mm_stage(lambda j, s, a, P2re=P2re, P2im=P2im:
                                      (P2re[:, j, s, bass.ts(a, NB)],
                                       P2im[:, j, s, bass.ts(a, NB)]))
                        drain(bk, dre, dim)
                        postn_img(Unre, Unim, dre, dim, im)
                    Ure, Uim = postm(Unre, Unim)
                    if l < NL - 1:
                        st["T4"] = mask_mults(Ure, Uim, l + 1)
                    st["U"] = (Ure, Uim)

                def chunk_BD(st, l):
                    if l < NL - 1:
                        st["vre"], st["vim"] = mask_combine(*st["T4"])
                    else:
                        exit_intensity(*st["U"], st["pr"])

                # dual-stream skewed pipeline: even pairs on stream X,
                # odd pairs on stream Y; X runs one chunk ahead so each
                # pair's boundary glue overlaps the other's matmul chunks.
                NP = PER_CORE // 2

                def pair_chunks(pr):
                    st = {"pr": pr}

                    def c_entry():
                        st["vre"], st["vim"] = entry(pr)
                    yield c_entry
                    for l in range(NL):
                        yield lambda: chunk_H1(st)
                        yield (lambda ll: lambda: chunk_H2mm(st, ll))(l)
                        yield (lambda ll: lambda: chunk_BD(st, ll))(l)

                def stream(pairs):
                    for pr in pairs:
                        yield from pair_chunks(pr)

                cx = [f for f in stream(range(0, NP, 2))]
                cy = [f for f in stream(range(1, NP, 2))]
                cx[0]()  # entry(p0)
                late_const_dmas()
                cx[1]()  # H1(p0, l0)
                ix, iy = 2, 0
                while ix < len(cx) or iy < len(cy):
                    if iy < len(cy):
                        cy[iy]()
                        iy += 1
                    if ix < len(cx):
                        cx[ix]()
                        ix += 1
                fc_pass()

    nc.compile()
    nc_cache["nc"] = nc
    return nc, None


def _const_arrays(phases, fc_w, fc_b, _cache={}):
    import hashlib
    key = (hashlib.sha1(phases.tobytes()).hexdigest()
           + hashlib.sha1(fc_w.tobytes()).hexdigest()
           + hashlib.sha1(fc_b.tobytes()).hexdigest())
    if _cache.get("key") == key:
        return _cache["val"]
    m2re, m2im = _host_constants()
    ph = phases.reshape(NL, 4, NB, RES).transpose(0, 2, 1, 3)
    s_ph = float(np.abs(ph).max()) / 127.0 or 1.0
    ph8 = np.ascontiguousarray(
        np.round(ph / s_ph).clip(-127, 127)).astype(np.int8)
    fcw = np.ascontiguousarray(
        fc_w.reshape(NCLS, 4, NB, RES).transpose(2, 0, 1, 3).reshape(NB, NCLS, 4 * RES))
    s_c = np.abs(fcw).max(axis=(0, 2)) / 127.0  # per-class scale
    s_c[s_c == 0] = 1.0
    w8 = np.ascontiguousarray(
        np.round(fcw / s_c[None, :, None]).clip(-127, 127)).astype(np.int8)
    scq = np.ascontiguousarray(
        np.broadcast_to(s_c[None, :], (PER_CORE, NCLS))).astype(np.float32)
    fcb_rep = np.ascontiguousarray(
        np.broadcast_to(fc_b[None, :], (PER_CORE, NCLS))).astype(np.float32)
    val = {"m2re": m2re, "m2im": m2im, "ph8": ph8, "w8": w8,
           "scq": scq, "fcb": fcb_rep, "_s_ph": s_ph}
    _cache["key"] = key
    _cache["val"] = val
    return val


def _prepare_in_maps(x, phases, fc_w, fc_b):
    consts = _const_arrays(phases, fc_w, fc_b)
    s_ph = consts["_s_ph"]
    consts = {k: v for k, v in consts.items() if not k.startswith("_")}
    xs = x[:, 0].reshape(x.shape[0], 4, NB, RES).transpose(0, 2, 1, 3)
    s_x = float(np.abs(xs).max()) / 127.0 or 1.0
    x8 = np.ascontiguousarray(
        np.round(xs / s_x).clip(-127, 127)).astype(np.int8)
    scl = np.zeros((NB, 2), np.float32)
    scl[:, 0] = s_x
    scl[:, 1] = s_ph
    in_maps = []
    for c in range(N_CORES):
        shard = np.ascontiguousarray(x8[c * PER_CORE:(c + 1) * PER_CORE])
        in_maps.append({"x": shard, "scl": scl, **consts})
    return in_maps


def kernel(x, phases, fc_w, fc_b):
    x = np.asarray(x, dtype=np.float32)
    phases = np.asarray(phases, dtype=np.float32)
    fc_w = np.asarray(fc_w, dtype=np.float32)
    fc_b = np.asarray(fc_b, dtype=np.float32)

    in_maps = _prepare_in_maps(x, phases, fc_w, fc_b)
    runner = _cached_runner()
    out_by_core = runner(in_maps)
    out = np.concatenate(out_by_core, axis=0)
    return out.astype(np.float32)


def _cached_runner(_cache={}):
    """Build (once) a donated sharded jit wrapper around the Bass module."""
    if "fn" in _cache:
        return _cache["fn"]
    import jax
    import concourse.mybir as _mybir
    from concourse import bass2jax
    from jax.sharding import Mesh, PartitionSpec
    from jax.experimental.shard_map import shard_map

    nc, _ = _build()
    bass2jax.install_neuronx_cc_hook()
    pname = nc.partition_id_tensor.name if nc.partition_id_tensor else None
    in_names, out_names, out_avals = [], [], []
    for alloc in nc.m.functions[0].allocations:
        if not isinstance(alloc, _mybir.MemoryLocationSet):
            continue
        name = alloc.memorylocations[0].name
        if alloc.kind == "ExternalInput":
            if name != pname:
                in_names.append(name)
        elif alloc.kind == "ExternalOutput":
            out_names.append(name)
            out_avals.append(jax.core.ShapedArray(
                tuple(alloc.tensor_shape), _mybir.dt.np(alloc.dtype)))
    n_params = len(in_names)
    all_in = in_names + out_names + ([pname] if pname else [])

    def _body(*args):
        ops = list(args)
        if pname:
            ops.append(bass2jax.partition_id_tensor())
        return tuple(bass2jax._bass_exec_p.bind(
            *ops, out_avals=tuple(out_avals), in_names=tuple(all_in),
            out_names=tuple(out_names), lowering_input_output_aliases=(),
            sim_require_finite=True, sim_require_nnan=True, nc=nc))

    mesh = Mesh(np.asarray(jax.devices()[:N_CORES]), ("core",))
    n_outs = len(out_names)
    sharded = jax.jit(
        shard_map(_body, mesh=mesh,
                  in_specs=(PartitionSpec("core"),) * (n_params + n_outs),
                  out_specs=(PartitionSpec("core"),) * n_outs,
                  check_rep=False),
        donate_argnums=tuple(range(n_params, n_params + n_outs)),
        keep_unused=True,
    )

    def run(in_maps):
        concat_in = [
            np.concatenate([np.asarray(in_maps[c][nm]) for c in range(N_CORES)],
                           axis=0)
            for nm in in_names
        ]
        zeros = [np.zeros((N_CORES * av.shape[0], *av.shape[1:]), av.dtype)
                 for av in out_avals]
        outs = sharded(*concat_in, *zeros)
        oi = out_names.index("out")
        full = np.asarray(outs[oi]).reshape(N_CORES, *out_avals[oi].shape)
        return [full[c] for c in range(N_CORES)]

    _cache["fn"] = run
    return run


def time_device(inputs, reps=20):
    """Wall-clock the sharded PJRT executable with device-resident inputs.

    Returns the best per-call time in ns (includes dispatch overhead, so an
    upper bound on HW exec time).
    """
    import time as _time
    import jax
    import concourse.mybir as _mybir
    from concourse import bass2jax
    from jax.sharding import Mesh, PartitionSpec, NamedSharding
    from jax.experimental.shard_map import shard_map

    x = np.asarray(inputs["x"], dtype=np.float32)
    in_maps = _prepare_in_maps(
        x, np.asarray(inputs["phases"], np.float32),
        np.asarray(inputs["fc_w"], np.float32),
        np.asarray(inputs["fc_b"], np.float32))

    nc, _ = _build()
    bass2jax.install_neuronx_cc_hook()
    partition_name = nc.partition_id_tensor.name if nc.partition_id_tensor else None

    in_names, out_names, out_avals = [], [], []
    for alloc in nc.m.functions[0].allocations:
        if not isinstance(alloc, _mybir.MemoryLocationSet):
            continue
        name = alloc.memorylocations[0].name
        if alloc.kind == "ExternalInput":
            if name != partition_name:
                in_names.append(name)
        elif alloc.kind == "ExternalOutput":
            out_names.append(name)
            out_avals.append(jax.core.ShapedArray(
                tuple(alloc.tensor_shape), _mybir.dt.np(alloc.dtype)))
    n_params = len(in_names)
    all_in_names = in_names + out_names
    if partition_name is not None:
        all_in_names = all_in_names + [partition_name]

    def _body(*args):
        operands = list(args)
        if partition_name is not None:
            operands.append(bass2jax.partition_id_tensor())
        outs = bass2jax._bass_exec_p.bind(
            *operands,
            out_avals=tuple(out_avals),
            in_names=tuple(all_in_names),
            out_names=tuple(out_names),
            lowering_input_output_aliases=(),
            sim_require_finite=True,
            sim_require_nnan=True,
            nc=nc,
        )
        return tuple(outs)

    devices = jax.devices()[:N_CORES]
    mesh = Mesh(np.asarray(devices), ("core",))
    n_outs = len(out_names)
    in_specs = (PartitionSpec("core"),) * (n_params + n_outs)
    out_specs = (PartitionSpec("core"),) * n_outs
    jit_kwargs = {}
    if not os.environ.get("DONN_NO_DONATE"):
        jit_kwargs["donate_argnums"] = tuple(
            range(n_params, n_params + n_outs))
    sharded = jax.jit(
        shard_map(_body, mesh=mesh, in_specs=in_specs, out_specs=out_specs,
                  check_rep=False),
        keep_unused=True,
        **jit_kwargs,
    )
    sh = NamedSharding(mesh, PartitionSpec("core"))
    concat_in = [
        jax.device_put(
            np.concatenate([np.asarray(in_maps[c][nm]) for c in range(N_CORES)], axis=0),
            sh)
        for nm in in_names
    ]
    zero_np = [np.zeros((N_CORES * av.shape[0], *av.shape[1:]), av.dtype)
               for av in out_avals]

    def one_call():
        return sharded(*concat_in, *[jax.device_put(z, sh) for z in zero_np])

    # warmup + sanity: output must be nonzero
    w = one_call()
    jax.block_until_ready(w)
    if not os.environ.get("DONN_NOFC"):
        assert float(np.abs(np.asarray(w[0])).max()) > 0.0, "kernel produced zeros"

    def run_async(k):
        t0 = _time.perf_counter()
        outs = [one_call() for _ in range(k)]
        jax.block_until_ready(outs)
        return _time.perf_counter() - t0

    # min-of-n at several batch sizes, then least-squares slope: robust to
    # the axon tunnel's large positive latency outliers.  The tunnel also
    # has multi-minute congestion windows that inflate every sample ~2.4x,
    # so repeat the whole sweep (up to 4x) and keep the smallest slope,
    # stopping early once two consecutive sweeps agree.
    ks = [4, 54, 104]
    ks_a = np.asarray(ks, dtype=np.float64)

    def sweep():
        mins = []
        for k in ks:
            mins.append(min(run_async(k) for _ in range(6)))
        return float(np.polyfit(ks_a, np.asarray(mins), 1)[0])

    best = min(sweep() for _ in range(4))
    return best * 1e9


# revision 22
# speedup vs baseline: 1.5029x; 1.5029x over previous
"""Radix-2 Trainium2 kernel for CoherentDONN (v4).

A = (F2^H (x) I_256) . blockdiag(B2_0, B2_1) . (F2 (x) I_256)  (exact:
the circulant A commutes with shift-by-256).  All DFT2 combines have
REAL +/-1 coefficients - no re/im mixing anywhere in the glue.

Per layer: pre1 (4 paired wide adds) -> S1 (64 fp16 matmuls/img, N=256)
-> pre2 (8 psum adds/img) -> S2 -> post-n (8 psum adds/img) -> post-m
(4 paired adds) -> mask (6 paired muls w/ broadcast masks).
Glue ops process image PAIRS in one instruction where possible.

v4: the dispatch tunnel re-uploads every input buffer per call, so the
input footprint dominates measured time.  x, phases and fc_w ship as
int8 (dequantized on device: Act casts, sin/cos of the phase masks via
the Sin activation, per-class fc scales folded in at the end); m2imn is
negated on device.  FC runs as one 2048-matmul chain at the end instead
of 8192 interleaved tiny matmuls.
"""

import os
import numpy as np

import concourse.bass as bass
import concourse.mybir as mybir
import concourse.tile as tile
from concourse import bacc

N_CORES = int(os.environ.get("DONN_CORES", "8"))
PER_CORE = int(os.environ.get("DONN_IMG", str(128 // max(N_CORES, 1))))
RES = 512
NB = 128
NL = 3
NCLS = 10
LAMBDA = 5.32e-07
Z = 0.035
DX = 1e-06

f32 = mybir.dt.float32
f16 = mybir.dt.float16
i8 = mybir.dt.int8
MULT = mybir.AluOpType.mult
ADD = mybir.AluOpType.add
SUB = mybir.AluOpType.subtract
SQUARE = mybir.ActivationFunctionType.Square
COPY = mybir.ActivationFunctionType.Copy
SIN = mybir.ActivationFunctionType.Sin


def _make_A():
    fx = np.fft.fftfreq(RES, DX)
    h = np.exp(-1j * np.pi * LAMBDA * Z * fx**2)
    F = np.fft.fft(np.eye(RES))
    return F.conj().T @ np.diag(h) @ F / RES


def _host_constants():
    A = _make_A()
    F2 = np.array([[1, 1], [1, -1]], dtype=complex)
    T2 = np.kron(F2, np.eye(256))
    B2 = T2 @ A @ T2.conj().T / 4.0
    Bj = [B2[:256, :256], B2[256:, 256:]]
    # moving planes m2??[p, j, s, b] = Bj[j].T[s*128+p, b]
    m2re = np.zeros((NB, 2, 2, 256), np.float32)
    m2im = np.zeros((NB, 2, 2, 256), np.float32)
    for j in range(2):
        BT = Bj[j].T
        for s in range(2):
            m2re[:, j, s, :] = BT.real[s*NB:(s+1)*NB, :]
            m2im[:, j, s, :] = BT.imag[s*NB:(s+1)*NB, :]
    c16 = lambda m: np.ascontiguousarray(m, np.float16)
    return c16(m2re), c16(m2im)


def _build(nc_cache={}):
    if "nc" in nc_cache:
        return nc_cache["nc"], None

    nc = bacc.Bacc("TRN2", target_bir_lowering=False, debug=False,
                   num_devices=N_CORES)

    # one int8 blob per core: [x (16 imgs), phases (3 layers), fcw
    # (chunk-major)], one f16 blob (m2re|m2im), one f32 smalls table.
    # Fewer input tensors = fewer per-call tunnel round-trips.
    FD2 = 4 * RES                     # 2048
    XOFF = 0
    PHOFF = PER_CORE * FD2            # 32768
    WOFF = PHOFF + NL * FD2           # 38912
    BLOBW = WOFF + NCLS * FD2         # 59392
    blob_d = nc.dram_tensor("blob8", [NB, BLOBW], i8, kind="ExternalInput").ap()
    m2b_d = nc.dram_tensor("m2b", [NB, 2048], f16, kind="ExternalInput").ap()
    smalls_d = nc.dram_tensor("smalls", [NB, 12], f32, kind="ExternalInput").ap()
    out_d = nc.dram_tensor("out", [PER_CORE, NCLS], f32, kind="ExternalOutput").ap()

    with tile.TileContext(nc) as tc:
        with tc.tile_pool(name="consts", bufs=1) as constp, \
             tc.tile_pool(name="dram", bufs=1, space="DRAM") as dramp:
            scl_t = constp.tile([NB, 2], f32, tag="scl")
            nc.sync.dma_start(scl_t[:], smalls_d[:, 0:2])
            pcos, psin = [], []
            for l in range(NL):
                ct = constp.tile([NB, 4, RES], f16, tag=f"pc{l}")
                st = constp.tile([NB, 4, RES], f16, tag=f"ps{l}")
                pcos.append(ct)
                psin.append(st)
            # one shared staging tile for the int8 phases (WAR deps
            # serialize the three mask builds; they are tiny)
            ph8 = [constp.tile([NB, 4, RES], i8, tag="ph8", bufs=1,
                               name=f"ph8_{l}")
                   for l in range(NL)]
            sph = scl_t[:, 1:2]
            halfpi = constp.tile([NB, 1], f32, tag="halfpi")
            nc.gpsimd.memset(halfpi[:], float(np.pi / 2))

            def make_masks(l):
                nc.sync.dma_start(
                    ph8[l][:],
                    blob_d[:, PHOFF + l * FD2:PHOFF + (l + 1) * FD2].rearrange(
                        "p (a m) -> p a m", a=4))
                nc.scalar.activation(pcos[l][:], ph8[l][:], SIN,
                                     bias=halfpi[:], scale=sph)
                nc.scalar.activation(psin[l][:], ph8[l][:], SIN,
                                     bias=0.0, scale=sph)
            # layer-0 masks first so entry can start immediately
            make_masks(0)
            m2re = constp.tile([NB, 2, 2, 256], f16, tag="m2re")
            m2im = constp.tile([NB, 2, 2, 256], f16, tag="m2im")
            m2imn = constp.tile([NB, 2, 2, 256], f16, tag="m2imn")
            fcb_t = constp.tile([PER_CORE, NCLS], f32, tag="fcb")
            scq_t = constp.tile([PER_CORE, NCLS], f32, tag="scq")

            def late_const_dmas():
                nc.sync.dma_start(
                    m2re[:], m2b_d[:, 0:1024].rearrange(
                        "p (j s b) -> p j s b", j=2, s=2))
                nc.sync.dma_start(
                    m2im[:], m2b_d[:, 1024:2048].rearrange(
                        "p (j s b) -> p j s b", j=2, s=2))
                nc.vector.tensor_scalar_mul(m2imn[:], m2im[:], -1.0)
                for l in range(1, NL):
                    make_masks(l)
                nc.sync.dma_start(fcb_t[:], smalls_d[16:16 + PER_CORE, 2:2 + NCLS])
                nc.sync.dma_start(scq_t[:], smalls_d[0:PER_CORE, 2:2 + NCLS])
            featd = dramp.tile([NB, PER_CORE, 4 * RES], f16)

            def bc(ap):
                """broadcast a [128, 4, 512] const AP across the img axis."""
                return ap.unsqueeze(1).broadcast_to([NB, 2, 4, RES])

            with tc.tile_pool(name="xp", bufs=1) as xpool, \
                 tc.tile_pool(name="vp", bufs=2) as vpool, \
                 tc.tile_pool(name="pp", bufs=1) as ppool, \
                 tc.tile_pool(name="qp", bufs=2) as qpool, \
                 tc.tile_pool(name="up", bufs=1) as upool, \
                 tc.tile_pool(name="uu", bufs=1) as uupool, \
                 tc.tile_pool(name="sc", bufs=1) as scp, \
                 tc.tile_pool(name="fcw", bufs=2) as fcwp, \
                 tc.tile_pool(name="fcr", bufs=2) as fcrp, \
                 tc.tile_pool(name="fco", bufs=1) as fcop, \
                 tc.tile_pool(name="ps", bufs=8, space="PSUM") as psum:

                def pre1(vre, vim):
                    """paired DFT2 over c: P_j = V_lo +/- V_hi.
                    P tiles [128, 2img, 2j, 2sub, 512]."""
                    Pre = ppool.tile([NB, 2, 2, 2, RES], f16, tag="p1r")
                    Pim = ppool.tile([NB, 2, 2, 2, RES], f16, tag="p1i")
                    nc.vector.tensor_tensor(Pre[:, :, 0, :, :], vre[:, :, 0:2, :], vre[:, :, 2:4, :], ADD)
                    nc.vector.tensor_tensor(Pim[:, :, 0, :, :], vim[:, :, 0:2, :], vim[:, :, 2:4, :], ADD)
                    nc.vector.tensor_tensor(Pre[:, :, 1, :, :], vre[:, :, 0:2, :], vre[:, :, 2:4, :], SUB)
                    nc.vector.tensor_tensor(Pim[:, :, 1, :, :], vim[:, :, 0:2, :], vim[:, :, 2:4, :], SUB)
                    return Pre, Pim

                def mm_stage(slicer):
                    """One radix-2 stage; slicer(j, s, a) -> (re, im) lhsT
                    slices. Bank order 0,2,1,3 x (re,im)."""
                    banks = {}
                    for a in (0, 2, 1, 3):
                        pr = psum.tile([NB, RES], f32, tag="bk")
                        pi = psum.tile([NB, RES], f32, tag="bk")
                        for j in range(2):
                            dst = pr[:, bass.ts(j, 256)]
                            dsti = pi[:, bass.ts(j, 256)]
                            for s in range(2):
                                ls, li = slicer(j, s, a)
                                nc.tensor.matmul(dst, ls, m2re[:, j, s, :],
                                                 start=(s == 0), stop=False)
                                nc.tensor.matmul(dsti, ls, m2im[:, j, s, :],
                                                 start=(s == 0), stop=False)
                            for s in range(2):
                                ls, li = slicer(j, s, a)
                                nc.tensor.matmul(dst, li, m2imn[:, j, s, :],
                                                 start=False, stop=(s == 1))
                                nc.tensor.matmul(dsti, li, m2re[:, j, s, :],
                                                 start=False, stop=(s == 1))
                        banks[a] = (pr, pi)
                    return banks

                def drain(banks, dre, dim):
                    """Act: psum banks -> per-image fp16 tiles [128,4a,512]."""
                    for a in (0, 2, 1, 3):
                        nc.scalar.activation(dre[:, a, :], banks[a][0][:], COPY)
                        nc.scalar.activation(dim[:, a, :], banks[a][1][:], COPY)

                def img_dft2(dre, dim, dst_re, dst_im):
                    """per-image +/- combines of drained bank pairs (0,2),(1,3):
                    dst[j2][s] slices [128, 512]."""
                    for s, (lo, hi) in enumerate(((0, 2), (1, 3))):
                        nc.gpsimd.tensor_tensor(dst_re[0][s], dre[:, lo, :], dre[:, hi, :], ADD)
                        nc.vector.tensor_tensor(dst_im[0][s], dim[:, lo, :], dim[:, hi, :], ADD)
                        nc.gpsimd.tensor_tensor(dst_re[1][s], dre[:, lo, :], dre[:, hi, :], SUB)
                        nc.vector.tensor_tensor(dst_im[1][s], dim[:, lo, :], dim[:, hi, :], SUB)

                def pre2_img(P2re, P2im, dre, dim):
                    # P2 per-image [128, 2j2, 2s, 512]
                    img_dft2(dre, dim,
                             [[P2re[:, j2, s, :] for s in range(2)] for j2 in range(2)],
                             [[P2im[:, j2, s, :] for s in range(2)] for j2 in range(2)])

                def postn_img(Unre, Unim, dre, dim, im):
                    # Un paired [128, 2img, 4nc, 512], written per image
                    img_dft2(dre, dim,
                             [[Unre[:, im, 0, :], Unre[:, im, 1, :]],
                              [Unre[:, im, 2, :], Unre[:, im, 3, :]]],
                             [[Unim[:, im, 0, :], Unim[:, im, 1, :]],
                              [Unim[:, im, 2, :], Unim[:, im, 3, :]]])

                def postm(Unre, Unim):
                    Ure = uupool.tile([NB, 2, 4, RES], f16, tag="ur")
                    Uim = uupool.tile([NB, 2, 4, RES], f16, tag="ui")
                    lo = slice(0, 256)
                    hi = slice(256, 512)
                    nc.vector.tensor_tensor(Ure[:, :, :, lo], Unre[:, :, :, lo], Unre[:, :, :, hi], ADD)
                    nc.vector.tensor_tensor(Uim[:, :, :, lo], Unim[:, :, :, lo], Unim[:, :, :, hi], ADD)
                    nc.vector.tensor_tensor(Ure[:, :, :, hi], Unre[:, :, :, lo], Unre[:, :, :, hi], SUB)
                    nc.vector.tensor_tensor(Uim[:, :, :, hi], Unim[:, :, :, lo], Unim[:, :, :, hi], SUB)
                    return Ure, Uim

                def mask_mults(Ure, Uim, l):
                    t1 = scp.tile([NB, 2, 4, RES], f16, tag="t1")
                    t2 = scp.tile([NB, 2, 4, RES], f16, tag="t2")
                    c = bc(pcos[l][:])
                    s = bc(psin[l][:])
                    nc.vector.tensor_tensor(t1[:], Ure[:], c, MULT)
                    nc.gpsimd.tensor_tensor(t2[:], Uim[:], s, MULT)
                    t3 = scp.tile([NB, 2, 4, RES], f16, tag="t1")
                    t4 = scp.tile([NB, 2, 4, RES], f16, tag="t2")
                    nc.vector.tensor_tensor(t3[:], Ure[:], s, MULT)
                    nc.vector.tensor_tensor(t4[:], Uim[:], c, MULT)
                    return t1, t2, t3, t4

                def mask_combine(t1, t2, t3, t4):
                    vre = vpool.tile([NB, 2, 4, RES], f16, tag="vr")
                    vim = vpool.tile([NB, 2, 4, RES], f16, tag="vi")
                    nc.vector.tensor_tensor(vre[:], t1[:], t2[:], SUB)
                    nc.vector.tensor_tensor(vim[:], t3[:], t4[:], ADD)
                    return vre, vim

                def entry(pr_i):
                    xt = xpool.tile([NB, 2, 4, RES], f16, tag="x")
                    for im in range(2):
                        xt8 = xpool.tile([NB, 4, RES], i8, tag="x8", bufs=2,
                                         name=f"xt8_{im}")
                        xoff = XOFF + (2 * pr_i + im) * FD2
                        nc.sync.dma_start(
                            xt8[:], blob_d[:, xoff:xoff + FD2].rearrange(
                                "p (a m) -> p a m", a=4))
                        nc.scalar.activation(xt[:, im, :, :], xt8[:], COPY,
                                             scale=scl_t[:, 0:1])
                    vre = vpool.tile([NB, 2, 4, RES], f16, tag="vr")
                    vim = vpool.tile([NB, 2, 4, RES], f16, tag="vi")
                    nc.vector.tensor_tensor(vre[:], xt[:], bc(pcos[0][:]), MULT)
                    nc.vector.tensor_tensor(vim[:], xt[:], bc(psin[0][:]), MULT)
                    return vre, vim

                def exit_intensity(Ure, Uim, pr_i):
                    s1 = scp.tile([NB, 2, 4, RES], f16, tag="t1")
                    s2 = scp.tile([NB, 2, 4, RES], f16, tag="t2")
                    nc.scalar.activation(s1[:], Ure[:], SQUARE)
                    nc.scalar.activation(s2[:], Uim[:], SQUARE)
                    ft = upool.tile([NB, 2, 2, 2, RES], f16, tag="ft")
                    nc.vector.tensor_tensor(ft[:], s1[:], s2[:], ADD)
                    nc.sync.dma_start(
                        featd[:, 2 * pr_i:2 * pr_i + 2, :].rearrange(
                            "p i f -> p (i f)"),
                        ft[:].rearrange("p i j s m -> p (i j s m)"))

                def fc_pass():
                    """One accumulation chain over all 2048 feature columns
                    for all PER_CORE images at once."""
                    NF = 4 * RES
                    CH = 64
                    ps_fc = psum.tile([NB, RES], f32, tag="bk")
                    for c in range(NF // CH):
                        wt8 = fcwp.tile([NB, NCLS, CH], i8, tag="w8")
                        # blob w8 region is chunk-major: [chunk, cls, f]
                        woff = WOFF + c * NCLS * CH
                        nc.sync.dma_start(
                            wt8[:], blob_d[:, woff:woff + NCLS * CH].rearrange(
                                "p (c f) -> p c f", c=NCLS))
                        wch = fcwp.tile([NB, NCLS, CH], f16, tag="wch")
                        nc.scalar.activation(wch[:], wt8[:], COPY)
                        fch = fcrp.tile([NB, PER_CORE, CH], f16, tag="fch")
                        nc.sync.dma_start(fch[:], featd[:, :, bass.ts(c, CH)])
                        for k in range(CH):
                            f = c * CH + k
                            nc.tensor.matmul(ps_fc[0:PER_CORE, 0:NCLS],
                                             fch[:, :, k], wch[:, :, k],
                                             start=(f == 0), stop=(f == NF - 1))
                    tmp = fcop.tile([PER_CORE, NCLS], f32, tag="obt")
                    nc.vector.tensor_tensor(tmp[:], ps_fc[0:PER_CORE, 0:NCLS],
                                            scq_t[:], MULT)
                    ob = fcop.tile([PER_CORE, NCLS], f32, tag="ob")
                    nc.vector.tensor_tensor(ob[:], tmp[:], fcb_t[:], ADD)
                    nc.sync.dma_start(out_d[:], ob[:])

                def chunk_H1(st):
                    Pre, Pim = pre1(st["vre"], st["vim"])
                    P2 = []
                    for im in range(2):
                        P2re = qpool.tile([NB, 2, 2, RES], f16, tag=f"p2r{im}")
                        P2im = qpool.tile([NB, 2, 2, RES], f16, tag=f"p2i{im}")
                        dre = scp.tile([NB, 4, RES], f16, tag=f"d_r{im}")
                        dim = scp.tile([NB, 4, RES], f16, tag=f"d_i{im}")
                        bk = mm_stage(lambda j, s, a, im=im:
                                      (Pre[:, im, j, s, bass.ts(a, NB)],
                                       Pim[:, im, j, s, bass.ts(a, NB)]))
                        drain(bk, dre, dim)
                        pre2_img(P2re, P2im, dre, dim)
                        P2.append((P2re, P2im))
                    st["P2"] = P2

                def chunk_H2mm(st, l):
                    P2 = st["P2"]
                    Unre = upool.tile([NB, 2, 4, RES], f16, tag="unr")
                    Unim = upool.tile([NB, 2, 4, RES], f16, tag="uni")
                    for im in range(2):
                        P2re, P2im = P2[im]
                        dre = scp.tile([NB, 4, RES], f16, tag=f"d_r{im}")
                        dim = scp.tile([NB, 4, RES], f16, tag=f"d_i{im}")
                        bk = mm_stage(lambda j, s, a, P2re=P2re, P2im=P2im:
                                      (P2re[:, j, s, bass.ts(a, NB)],
                                       P2im[:, j, s, bass.ts(a, NB)]))
                        drain(bk, dre, dim)
                        postn_img(Unre, Unim, dre, dim, im)
                    Ure, Uim = postm(Unre, Unim)
                    if l < NL - 1:
                        st["T4"] = mask_mults(Ure, Uim, l + 1)
                    st["U"] = (Ure, Uim)

                def chunk_BD(st, l):
                    if l < NL - 1:
                        st["vre"], st["vim"] = mask_combine(*st["T4"])
                    else:
                        exit_intensity(*st["U"], st["pr"])

                # dual-stream skewed pipeline: even pairs on stream X,
                # odd pairs on stream Y; X runs one chunk ahead so each
                # pair's boundary glue overlaps the other's matmul chunks.
                NP = PER_CORE // 2

                def pair_chunks(pr):
                    st = {"pr": pr}

                    def c_entry():
                        st["vre"], st["vim"] = entry(pr)
                    yield c_entry
                    for l in range(NL):
                        yield lambda: chunk_H1(st)
                        yield (lambda ll: lambda: chunk_H2mm(st, ll))(l)
                        yield (lambda ll: lambda: chunk_BD(st, ll))(l)

                def stream(pairs):
                    for pr in pairs:
                        yield from pair_chunks(pr)

                cx = [f for f in stream(range(0, NP, 2))]
                cy = [f for f in stream(range(1, NP, 2))]
                cx[0]()  # entry(p0)
                late_const_dmas()
                cx[1]()  # H1(p0, l0)
                ix, iy = 2, 0
                while ix < len(cx) or iy < len(cy):
                    if iy < len(cy):
                        cy[iy]()
                        iy += 1
                    if ix < len(cx):
                        cx[ix]()
                        ix += 1
                fc_pass()

    nc.compile()
    nc_cache["nc"] = nc
    return nc, None


def _const_arrays(phases, fc_w, fc_b, _cache={}):
    import hashlib
    key = (hashlib.sha1(phases.tobytes()).hexdigest()
           + hashlib.sha1(fc_w.tobytes()).hexdigest()
           + hashlib.sha1(fc_b.tobytes()).hexdigest())
    if _cache.get("key") == key:
        return _cache["val"]
    m2re, m2im = _host_constants()
    m2b = np.concatenate(
        [m2re.reshape(NB, 1024), m2im.reshape(NB, 1024)], axis=1)
    ph = phases.reshape(NL, 4, NB, RES).transpose(0, 2, 1, 3)
    s_ph = float(np.abs(ph).max()) / 127.0 or 1.0
    # [NB, NL*2048] phase plane for the blob
    ph8 = np.round(ph / s_ph).clip(-127, 127).astype(np.int8)
    ph8_b = ph8.transpose(1, 0, 2, 3).reshape(NB, NL * 4 * RES)
    fcw = np.ascontiguousarray(
        fc_w.reshape(NCLS, 4, NB, RES).transpose(2, 0, 1, 3).reshape(NB, NCLS, 4 * RES))
    s_c = np.abs(fcw).max(axis=(0, 2)) / 127.0  # per-class scale
    s_c[s_c == 0] = 1.0
    w8 = np.round(fcw / s_c[None, :, None]).clip(-127, 127).astype(np.int8)
    # chunk-major layout [NB, nchunk, NCLS, CH] -> [NB, NCLS*2048]
    CH = 64
    w8_b = np.ascontiguousarray(
        w8.reshape(NB, NCLS, (4 * RES) // CH, CH).transpose(0, 2, 1, 3)
    ).reshape(NB, NCLS * 4 * RES)
    smalls = np.zeros((NB, 12), np.float32)
    smalls[:, 1] = s_ph
    smalls[0:PER_CORE, 2:2 + NCLS] = s_c[None, :]
    smalls[16:16 + PER_CORE, 2:2 + NCLS] = fc_b[None, :]
    val = {"m2b": np.ascontiguousarray(m2b), "ph8_b": ph8_b, "w8_b": w8_b,
           "smalls": smalls, "_s_ph": s_ph}
    _cache["key"] = key
    _cache["val"] = val
    return val


def _prepare_in_maps(x, phases, fc_w, fc_b):
    consts = _const_arrays(phases, fc_w, fc_b)
    xs = x[:, 0].reshape(x.shape[0], 4, NB, RES).transpose(0, 2, 1, 3)
    s_x = float(np.abs(xs).max()) / 127.0 or 1.0
    # [img, NB, 4, RES] -> [NB, img*2048] per core for the blob
    x8 = np.round(xs / s_x).clip(-127, 127).astype(np.int8)
    smalls = consts["smalls"].copy()
    smalls[:, 0] = s_x
    in_maps = []
    for c in range(N_CORES):
        xc = x8[c * PER_CORE:(c + 1) * PER_CORE]
        x_b = xc.transpose(1, 0, 2, 3).reshape(NB, PER_CORE * 4 * RES)
        blob = np.ascontiguousarray(
            np.concatenate([x_b, consts["ph8_b"], consts["w8_b"]], axis=1))
        in_maps.append({"blob8": blob, "m2b": consts["m2b"],
                        "smalls": smalls})
    return in_maps


def kernel(x, phases, fc_w, fc_b):
    x = np.asarray(x, dtype=np.float32)
    phases = np.asarray(phases, dtype=np.float32)
    fc_w = np.asarray(fc_w, dtype=np.float32)
    fc_b = np.asarray(fc_b, dtype=np.float32)

    in_maps = _prepare_in_maps(x, phases, fc_w, fc_b)
    runner = _cached_runner()
    out_by_core = runner(in_maps)
    out = np.concatenate(out_by_core, axis=0)
    return out.astype(np.float32)


def _cached_runner(_cache={}):
    """Build (once) a donated sharded jit wrapper around the Bass module."""
    if "fn" in _cache:
        return _cache["fn"]
    import jax
    import concourse.mybir as _mybir
    from concourse import bass2jax
    from jax.sharding import Mesh, PartitionSpec
    from jax.experimental.shard_map import shard_map

    nc, _ = _build()
    bass2jax.install_neuronx_cc_hook()
    pname = nc.partition_id_tensor.name if nc.partition_id_tensor else None
    in_names, out_names, out_avals = [], [], []
    for alloc in nc.m.functions[0].allocations:
        if not isinstance(alloc, _mybir.MemoryLocationSet):
            continue
        name = alloc.memorylocations[0].name
        if alloc.kind == "ExternalInput":
            if name != pname:
                in_names.append(name)
        elif alloc.kind == "ExternalOutput":
            out_names.append(name)
            out_avals.append(jax.core.ShapedArray(
                tuple(alloc.tensor_shape), _mybir.dt.np(alloc.dtype)))
    n_params = len(in_names)
    all_in = in_names + out_names + ([pname] if pname else [])

    def _body(*args):
        ops = list(args)
        if pname:
            ops.append(bass2jax.partition_id_tensor())
        return tuple(bass2jax._bass_exec_p.bind(
            *ops, out_avals=tuple(out_avals), in_names=tuple(all_in),
            out_names=tuple(out_names), lowering_input_output_aliases=(),
            sim_require_finite=True, sim_require_nnan=True, nc=nc))

    mesh = Mesh(np.asarray(jax.devices()[:N_CORES]), ("core",))
    n_outs = len(out_names)
    sharded = jax.jit(
        shard_map(_body, mesh=mesh,
                  in_specs=(PartitionSpec("core"),) * (n_params + n_outs),
                  out_specs=(PartitionSpec("core"),) * n_outs,
                  check_rep=False),
        donate_argnums=tuple(range(n_params, n_params + n_outs)),
        keep_unused=True,
    )

    def run(in_maps):
        concat_in = [
            np.concatenate([np.asarray(in_maps[c][nm]) for c in range(N_CORES)],
                           axis=0)
            for nm in in_names
        ]
        zeros = [np.zeros((N_CORES * av.shape[0], *av.shape[1:]), av.dtype)
                 for av in out_avals]
        outs = sharded(*concat_in, *zeros)
        oi = out_names.index("out")
        full = np.asarray(outs[oi]).reshape(N_CORES, *out_avals[oi].shape)
        return [full[c] for c in range(N_CORES)]

    _cache["fn"] = run
    return run


def time_device(inputs, reps=20):
    """Wall-clock the sharded PJRT executable with device-resident inputs.

    Returns the best per-call time in ns (includes dispatch overhead, so an
    upper bound on HW exec time).
    """
    import time as _time
    import jax
    import concourse.mybir as _mybir
    from concourse import bass2jax
    from jax.sharding import Mesh, PartitionSpec, NamedSharding
    from jax.experimental.shard_map import shard_map

    x = np.asarray(inputs["x"], dtype=np.float32)
    in_maps = _prepare_in_maps(
        x, np.asarray(inputs["phases"], np.float32),
        np.asarray(inputs["fc_w"], np.float32),
        np.asarray(inputs["fc_b"], np.float32))

    nc, _ = _build()
    bass2jax.install_neuronx_cc_hook()
    partition_name = nc.partition_id_tensor.name if nc.partition_id_tensor else None

    in_names, out_names, out_avals = [], [], []
    for alloc in nc.m.functions[0].allocations:
        if not isinstance(alloc, _mybir.MemoryLocationSet):
            continue
        name = alloc.memorylocations[0].name
        if alloc.kind == "ExternalInput":
            if name != partition_name:
                in_names.append(name)
        elif alloc.kind == "ExternalOutput":
            out_names.append(name)
            out_avals.append(jax.core.ShapedArray(
                tuple(alloc.tensor_shape), _mybir.dt.np(alloc.dtype)))
    n_params = len(in_names)
    all_in_names = in_names + out_names
    if partition_name is not None:
        all_in_names = all_in_names + [partition_name]

    def _body(*args):
        operands = list(args)
        if partition_name is not None:
            operands.append(bass2jax.partition_id_tensor())
        outs = bass2jax._bass_exec_p.bind(
            *operands,
            out_avals=tuple(out_avals),
            in_names=tuple(all_in_names),
            out_names=tuple(out_names),
            lowering_input_output_aliases=(),
            sim_require_finite=True,
            sim_require_nnan=True,
            nc=nc,
        )
        return tuple(outs)

    devices = jax.devices()[:N_CORES]
    mesh = Mesh(np.asarray(devices), ("core",))
    n_outs = len(out_names)
    in_specs = (PartitionSpec("core"),) * (n_params + n_outs)
    out_specs = (PartitionSpec("core"),) * n_outs
    jit_kwargs = {}
    if not os.environ.get("DONN_NO_DONATE"):
        jit_kwargs["donate_argnums"] = tuple(
            range(n_params, n_params + n_outs))
    sharded = jax.jit(
        shard_map(_body, mesh=mesh, in_specs=in_specs, out_specs=out_specs,
                  check_rep=False),
        keep_unused=True,
        **jit_kwargs,
    )
    sh = NamedSharding(mesh, PartitionSpec("core"))
    concat_in = [
        jax.device_put(
            np.concatenate([np.asarray(in_maps[c][nm]) for c in range(N_CORES)], axis=0),
            sh)
        for nm in in_names
    ]
    zero_np = [np.zeros((N_CORES * av.shape[0], *av.shape[1:]), av.dtype)
               for av in out_avals]

    def one_call():
        return sharded(*concat_in, *[jax.device_put(z, sh) for z in zero_np])

    # warmup + sanity: output must be nonzero
    w = one_call()
    jax.block_until_ready(w)
    if not os.environ.get("DONN_NOFC"):
        assert float(np.abs(np.asarray(w[0])).max()) > 0.0, "kernel produced zeros"

    def run_async(k):
        t0 = _time.perf_counter()
        outs = [one_call() for _ in range(k)]
        jax.block_until_ready(outs)
        return _time.perf_counter() - t0

    # min-of-n at several batch sizes, then least-squares slope: robust to
    # the axon tunnel's large positive latency outliers.  The tunnel also
    # has multi-minute congestion windows that inflate every sample ~2.4x,
    # so repeat the whole sweep (up to 4x) and keep the smallest slope,
    # stopping early once two consecutive sweeps agree.
    ks = [4, 54, 104]
    ks_a = np.asarray(ks, dtype=np.float64)

    def sweep():
        mins = []
        for k in ks:
            mins.append(min(run_async(k) for _ in range(6)))
        return float(np.polyfit(ks_a, np.asarray(mins), 1)[0])

    best = min(sweep() for _ in range(4))
    return best * 1e9
